# revision 18
# baseline (speedup 1.0000x reference)
"""GCNConv (N=100000, E=1600000, C=128) on 8 trn2 NeuronCores.

Sharding strategy (node-parallel, per the hint): destination nodes are
partitioned across the 8 cores, load-balanced (LPT bin-packing) into
128-row dest tiles. Edge routing is done on host as part of sharding:
edges are bucketed by destination tile and the per-edge source feature
rows (the "gathered source features" of the hint's all-to-all) are
materialized as a dest-sorted bf16 stream per core, from the
dis[col]-prescaled table x'' = diag(1/sqrt(deg)) @ x. The device then
does all the math: the segment_sum over each destination's messages
(PE selection-matmuls accumulating in PSUM), the W transform, and the
dis[row] output scaling.

Why no device-side per-edge gather: every dynamic-indexing mechanism on
trn2 (SWDGE indirect DMA, InstDMAGatherAnt, InstAPGather) was measured
at ~50 ns per row/descriptor per core (Q7 ucode rate), i.e. >10 ms for
1.7M edges -- 40x slower than streaming the routed messages at HBM rate.

Device pipeline per dest tile t (128 dests, K=17 chunks of 128 messages):
  msgs tile [128 msg, K*128 feat] <- one contiguous 557KB DMA (HWDGE)
  SelT[m, k*128+d] = (dlocal[m,k] == d)     one whole-tile DVE is_equal
  for chunk c: psum_sT[feat, dest] += msgs_c.T @ SelT_c     # PE, fp32
  sT -> SBUF bf16 (ACT copy)
  psum_out[dest, feat_out] = sT.T @ W                        # PE
  out_t = psum_out * disout   (ACT) -> DMA to HBM

Measured: ~250 us per pass on 8 cores (message stream 437MB bf16 at
~2.9TB/s aggregate + DVE sel builds, fully overlapped); output rel err
vs fp32 reference ~2.9e-3 (bf16 messages/weights, fp32 accumulation).
"""
import math

import numpy as np
import ml_dtypes

import concourse.bacc as bacc
import concourse.tile as tile
from concourse import mybir
from concourse.bass import AP
from concourse.bass_utils import run_bass_kernel_spmd

N_CORES = 8
P = 128

BF16 = ml_dtypes.bfloat16


def build_nc(n_tiles: int, K: int, repeat: int = 1, msgs_tiles=None,
             sel_mode='multi', copy_eng='scalar', disout_eng='scalar',
             gp_frac=0, bufs=None):
    """Build the SPMD Bass kernel: n_tiles dest tiles per core, K chunks of
    128 messages per tile.

    repeat>1 wraps the tile loop in a hardware For_i (idempotent re-run;
    timing only). msgs_tiles (timing only) shrinks the msgs input to that
    many tiles, read as msgs[t % msgs_tiles] -- same device work, tiny
    host->device transfer."""
    nc = bacc.Bacc("TRN2", target_bir_lowering=False, debug=False)
    T = n_tiles
    f32 = mybir.dt.float32
    bf16 = mybir.dt.bfloat16

    MT = msgs_tiles if msgs_tiles is not None else T
    b = {"msgp": 4, "selp": 8, "sTp": 3, "outp": 3, "psA": 4, "psB": 2}
    if bufs:
        b.update(bufs)
    msgs = nc.dram_tensor("msgs", [MT, P, K * P], bf16, kind="ExternalInput")
    dlocal = nc.dram_tensor("dlocal", [P, T * K], bf16, kind="ExternalInput")
    disout = nc.dram_tensor("disout", [P, T], f32, kind="ExternalInput")
    w16 = nc.dram_tensor("w16", [P, P], bf16, kind="ExternalInput")
    iota = nc.dram_tensor("iota", [P, P], bf16, kind="ExternalInput")
    if sel_mode == "ts":
        dlocal32 = nc.dram_tensor("dlocal32", [P, T * K], f32, kind="ExternalInput")
    out = nc.dram_tensor("out", [T * P, P], f32, kind="ExternalOutput")

    with tile.TileContext(nc) as tc:
        with tc.tile_pool(name="const", bufs=1) as constp, \
             tc.tile_pool(name="msgp", bufs=b["msgp"]) as msgp, \
             tc.tile_pool(name="selp", bufs=b["selp"]) as selp, \
             tc.tile_pool(name="sTp", bufs=b["sTp"]) as sTp, \
             tc.tile_pool(name="outp", bufs=b["outp"]) as outp, \
             tc.tile_pool(name="psA", bufs=b["psA"], space="PSUM") as psA, \
             tc.tile_pool(name="psB", bufs=b["psB"], space="PSUM") as psB:
            w_t = constp.tile([P, P], bf16)
            nc.sync.dma_start(w_t[:], w16[:])
            iota_t = constp.tile([P, P], bf16)
            nc.sync.dma_start(iota_t[:], iota[:])
            dlocal_t = constp.tile([P, T * K], bf16)
            nc.sync.dma_start(dlocal_t[:], dlocal[:])
            disout_t = constp.tile([P, T], f32)
            nc.sync.dma_start(disout_t[:], disout[:])
            if sel_mode == "ts":
                dlocal32_t = constp.tile([P, T * K], f32)
                nc.sync.dma_start(dlocal32_t[:], dlocal32[:])

            def body():
              for t in range(T):
                m_t = msgp.tile([P, K * P], bf16, tag="m")
                nc.sync.dma_start(m_t[:], msgs[t % MT])
                ps = psA.tile([P, P], f32, tag="psA")
                if sel_mode == "multi":
                    sel_m = selp.tile([P, K * P], bf16, tag="sel")
                    sel3 = sel_m[:].rearrange("p (k f) -> p k f", k=K)
                    dl_b = dlocal_t[:, t * K:(t + 1) * K].broadcast_to([P, K, P])
                    io = iota_t[:]
                    io3 = AP(io.tensor, io.offset, [[io.ap[0][0], P], [0, K], [1, P]])
                    eng = nc.gpsimd if (gp_frac and t % gp_frac == gp_frac - 1) \
                        else nc.vector
                    eng.tensor_tensor(
                        out=sel3, in0=dl_b, in1=io3, op=mybir.AluOpType.is_equal)
                    for c in range(K):
                        nc.tensor.matmul(
                            out=ps[:],
                            lhsT=m_t[:, c * P:(c + 1) * P],
                            rhs=sel_m[:, c * P:(c + 1) * P],
                            start=(c == 0),
                            stop=(c == K - 1),
                        )
                elif sel_mode == "ts":
                    for c in range(K):
                        col = t * K + c
                        sel = selp.tile([P, P], bf16, tag="sel")
                        nc.vector.tensor_scalar(
                            out=sel[:],
                            in0=iota_t[:],
                            scalar1=dlocal32_t[:, col:col + 1],
                            scalar2=None,
                            op0=mybir.AluOpType.is_equal,
                        )
                        nc.tensor.matmul(
                            out=ps[:],
                            lhsT=m_t[:, c * P:(c + 1) * P],
                            rhs=sel[:],
                            start=(c == 0),
                            stop=(c == K - 1),
                        )
                elif sel_mode == "none":
                    for c in range(K):
                        nc.tensor.matmul(
                            out=ps[:], lhsT=m_t[:, c * P:(c + 1) * P],
                            rhs=w_t[:], start=(c == 0), stop=(c == K - 1))
                else:
                    for c in range(K):
                        col = t * K + c
                        sel = selp.tile([P, P], bf16, tag="sel")
                        nc.vector.tensor_tensor(
                            out=sel[:],
                            in0=dlocal_t[:, col:col + 1].to_broadcast([P, P]),
                            in1=iota_t[:],
                            op=mybir.AluOpType.is_equal,
                        )
                        nc.tensor.matmul(
                            out=ps[:],
                            lhsT=m_t[:, c * P:(c + 1) * P],
                            rhs=sel[:],
                            start=(c == 0),
                            stop=(c == K - 1),
                        )
                sT = sTp.tile([P, P], bf16, tag="sT")
                if copy_eng == "scalar":
                    nc.scalar.copy(out=sT[:], in_=ps[:])
                else:
                    nc.vector.tensor_copy(out=sT[:], in_=ps[:])
                ps2 = psB.tile([P, P], f32, tag="psB")
                nc.tensor.matmul(out=ps2[:], lhsT=sT[:], rhs=w_t[:],
                                 start=True, stop=True)
                o_t = outp.tile([P, P], f32, tag="o")
                if disout_eng == "scalar":
                    nc.scalar.mul(o_t[:], ps2[:], disout_t[:, t:t + 1])
                else:
                    nc.vector.tensor_scalar_mul(o_t[:], ps2[:], disout_t[:, t:t + 1])
                nc.sync.dma_start(out[t * P:(t + 1) * P, :], o_t[:])
            if repeat == 1:
                body()
            else:
                with tc.For_i(0, repeat, 1):
                    body()
    nc.compile()
    return nc


def _route(x, W, edge_index, num_nodes, n_cores=N_CORES):
    """Host-side sharding/routing. Returns (in_maps, node_of, n_tiles, K)."""
    N = int(num_nodes)
    row = np.asarray(edge_index[0], dtype=np.int64)
    col = np.asarray(edge_index[1], dtype=np.int64)
    loops = np.arange(N, dtype=np.int64)
    row = np.concatenate([row, loops])
    col = np.concatenate([col, loops])
    E = row.shape[0]

    # symmetric degree normalization (degree counted on col, as reference)
    deg = np.bincount(col, minlength=N)
    dis = np.zeros(N, dtype=np.float32)
    nz = deg > 0
    dis[nz] = 1.0 / np.sqrt(deg[nz].astype(np.float64)).astype(np.float32)

    # --- load-balanced assignment of dest nodes to (core, tile, slot) ---
    deg_in = np.bincount(row, minlength=N)  # messages per dest
    n_tiles = math.ceil(N / (n_cores * P) / 1.0)
    n_tiles = math.ceil(N / n_cores / P)          # tiles per core
    TT = n_cores * n_tiles                        # total tiles
    # LPT: biggest dests first, into least-loaded tile with free slots
    import heapq
    order = np.argsort(-deg_in, kind="stable")
    heap = [(0, tt) for tt in range(TT)]
    heapq.heapify(heap)
    slots_used = np.zeros(TT, dtype=np.int64)
    tile_of = np.empty(N, dtype=np.int64)
    slot_of = np.empty(N, dtype=np.int64)
    spill = []
    for d in order:
        while True:
            load, tt = heapq.heappop(heap)
            if slots_used[tt] < P:
                break
            spill.append((load, tt))  # full tile: drop permanently
        tile_of[d] = tt
        slot_of[d] = slots_used[tt]
        slots_used[tt] += 1
        heapq.heappush(heap, (load + int(deg_in[d]), tt))

    # edges -> tiles, then slots within tile
    gt = tile_of[row]                             # tile of each edge
    e_order = np.argsort(gt, kind="stable")
    gt_s = gt[e_order]
    counts = np.bincount(gt_s, minlength=TT)
    K = int(math.ceil(counts.max() / P))
    starts = np.zeros(TT + 1, dtype=np.int64)
    np.cumsum(counts, out=starts[1:])
    pos = np.arange(E, dtype=np.int64) - starts[gt_s]
    c_e = pos // P
    m_e = pos % P

    x16 = (np.asarray(x, dtype=np.float32) * dis[:, None]).astype(BF16)

    msgs = np.zeros((TT, P, K, P), dtype=BF16)
    msgs[gt_s, m_e, c_e, :] = x16[col[e_order]]

    dlocal = np.full((TT, K, P), 255.0, dtype=BF16)
    dlocal[gt_s, c_e, m_e] = slot_of[row[e_order]].astype(BF16)

    disout = np.zeros((TT, P), dtype=np.float32)
    node_of = np.full((TT, P), -1, dtype=np.int64)
    node_of[tile_of, slot_of] = np.arange(N)
    valid = node_of >= 0
    disout[valid] = dis[node_of[valid]]

    w16 = np.asarray(W, dtype=np.float32).astype(BF16)
    iota = np.tile(np.arange(P, dtype=np.float32).astype(BF16), (P, 1))

    in_maps = []
    for cidx in range(n_cores):
        sl = slice(cidx * n_tiles, (cidx + 1) * n_tiles)
        # device dlocal layout: [P(m), T*K] with column t*K+c
        dl = np.ascontiguousarray(
            dlocal[sl].reshape(n_tiles * K, P).T)
        do = np.ascontiguousarray(disout[sl].T)     # [P(slot), T]
        in_maps.append({
            "msgs": np.ascontiguousarray(
                msgs[sl].reshape(n_tiles, P, K * P)),
            "dlocal": dl,
            "disout": do,
            "w16": w16,
            "iota": iota,
        })
    return in_maps, node_of, n_tiles, K


def kernel(x, W, edge_index, num_nodes):
    N = int(num_nodes)
    in_maps, node_of, n_tiles, K = _route(x, W, edge_index, N)
    nc = build_nc(n_tiles, K)
    try:
        res = run_bass_kernel_spmd(nc, in_maps, core_ids=list(range(N_CORES)))
    except Exception:
        # a previous process can leave a core wedged (NRT_EXEC_UNIT_
        # UNRECOVERABLE); one retry after the runtime re-initializes
        # reliably clears it.
        import time as _time
        _time.sleep(5.0)
        res = run_bass_kernel_spmd(nc, in_maps, core_ids=list(range(N_CORES)))
    C = np.asarray(W).shape[1]
    out = np.zeros((N, C), dtype=np.float32)
    TT = node_of.shape[0]
    per_core = TT // N_CORES
    outs = np.concatenate(
        [res.results[c]["out"].reshape(per_core, P, C) for c in range(N_CORES)],
        axis=0)                                    # [TT, P, C]
    valid = node_of >= 0
    out[node_of[valid]] = outs[valid]
    return out
